# revision 46
# baseline (speedup 1.0000x reference)
"""BoT-MHSA Trainium2 kernel (8-core SPMD, data-parallel over (batch, head) pairs).

Architecture (477us baseline -> ~373us):
  - fp16 PE datapath for x/w/Q/KP (11-bit mantissa keeps logit error ~3e-3;
    bf16 was not enough: at |logits|~30 its ulp is 0.125), bf16 for E/V/ctx.
  - Q and K^T projections write both PSUM partition halves via col-tiled
    matmul pairs (concurrent in the PE array): Q is replicated to rows
    64:128, KP keeps key-chunk A (c<128) in rows 0:64 / chunk B in 64:128.
  - QK is a 2-way ROW-TILED pair (contraction hs=64 per half): both key
    chunks compute concurrently -> L [128 keys, 1024] = [L_A(512q)|L_B(512q)].
  - E = exp(L) on ACT -> bf16 SBUF; ctx^T(+denom row) = [V|1]^T @ E
    accumulated over 32 key chunks in PSUM; transpose + 1/denom on PE/DVE.
  - Phase 2 is one flat software-pipelined loop over (pair, sb, m) with
    QK(k+1) emitted BEFORE PV(k): the in-order PE fills L(k+1) while ACT
    exps L(k), so the scalar engine (the binding resource: 33.5M exps/core
    at 1 elem/lane/cycle) runs back-to-back. Output stages are deferred one
    iteration; pair-1 projections interleave into early-loop PE slack.
  - Engine-clock self-waits (own-engine sem-ge) are stripped: ACT/PE writes
    land in issue order, so they only added sem-propagation stalls and
    1-wait-cap NoOp splits. (DVE self-waits are kept -- its dual-pipe
    datapath does reorder; dropping them corrupts results.)

Decomposition (per (b, n) of the 4x4 (batch, head) pairs -- 2 pairs per core):
  Tokens s = c*16 + t with head n = h%4, d = w (faithful-torch-view scramble):
    q[(c,t), w] = sum_ci wq[c, ci] x[b, ci, 4t+n, w] + bq[c]
  xs   = x[b, :, n::4, :]              [256, 1024]
  Full output assembled on host from the 16 per-pair slices.
"""

import sys

sys.path.insert(0, "/opt/trn_rl_repo")

import numpy as np

import concourse.bass as bass
import concourse.tile as tile
from concourse import mybir
from concourse.bass_utils import run_bass_kernel_spmd
from concourse.vector_clock import ScopedClock

F32 = mybir.dt.float32
F32R = mybir.dt.float32r
FP16 = mybir.dt.float16
BF16 = mybir.dt.bfloat16
N_CORES = 8
NH, HS = 4, 64
C, H, W = 256, 64, 64
S = H * W  # 4096
NT = 16  # t values per head (h = 4t + n)
PSUB = NT * W  # 1024 spatial positions per head subset

# ---------------------------------------------------------------------------
# Workaround: this container's walrus caps sync-waits per instruction at 1 for
# CTRL-class (Drain) instructions. Split the Tile tail-drain waits across
# follow-up SP nops (same engine, in order, before the all-engine barrier).
# ---------------------------------------------------------------------------
_MAX_WAITS = 1


def _split_waits(nc, inst):
    si = inst.sync_info
    if si is None:
        return
    waits = list(si.on_wait)
    if len(waits) <= _MAX_WAITS:
        return
    inst.sync_info = mybir.SyncInfo(
        on_wait=waits[:_MAX_WAITS], on_update=list(si.on_update)
    )
    for i in range(_MAX_WAITS, len(waits), _MAX_WAITS):
        nop = nc.sync.nop()
        nop.ins.sync_info = mybir.SyncInfo(on_wait=waits[i : i + _MAX_WAITS], on_update=[])


def _patched_drain_and_barrier(self, tick_clock, wait_clock):
    nc = self.nc
    drain_inst = nc.sync.drain()
    wait_clock.add_sem_waits(drain_inst.ins, ScopedClock({None: tick_clock.global_clock}))
    _split_waits(nc, drain_inst.ins)
    nc.all_engine_barrier()
    assert self.sems is not None
    popped = nc._tile_sem_poison_stack.pop()
    assert popped is self._sem_poison
    nc.clear_and_free_semaphores(list(self.sems.allocated().values()))
    nc.all_engine_barrier()


tile.TileContext._drain_and_barrier = _patched_drain_and_barrier


def _coalesce_waits(waits):
    """Merge sem-ge-imm waits on the same semaphore to a single max-threshold
    wait (sem-ge is monotone, so waiting for the max implies all of them)."""
    merged = {}
    out = []
    for w in waits:
        if w.wait_mode == "sem-ge-imm" and w.wait_reg is None:
            key = (w.sync_type, w.id)
            if key in merged:
                if w.wait_value > merged[key].wait_value:
                    merged[key].wait_value = w.wait_value
                continue
            merged[key] = w
        out.append(w)
    return out


# Dropping self-waits is only safe where the engine's writes land in issue
# order: true for ACT (single pipe, strict FIFO) and PE (matmuls pc-monotone
# in start AND end). NOT true for DVE (dual-pipe) -- verified empirically:
# dropping DVE self-waits corrupts results.
_ENGINE_SEM_PREFIX = {
    mybir.EngineType.Activation: "Activation_",
    mybir.EngineType.PE: "PE_",
}
# Self-waits stay on control-flow/drain instructions (their semantics depend
# on observing the engine's own completion count).
_KEEP_SELF_WAIT = {"InstDrain", "InstEventSemaphore", "InstAllEngineBarrier"}


def _drop_self_waits(inst, waits):
    """Remove sem-ge waits on the instruction's own engine clock: the engine
    executes in order and its matmuls/activations complete pc-monotonically,
    so program order already implies them. This avoids both the stall until
    the prior instruction's sem PROPAGATES and the 1-wait-cap NoOp split."""
    if type(inst).__name__ in _KEEP_SELF_WAIT:
        return waits
    pfx = _ENGINE_SEM_PREFIX.get(inst.engine)
    if pfx is None:
        return waits
    return [
        w
        for w in waits
        if not (
            w.wait_mode == "sem-ge-imm"
            and w.wait_reg is None
            and w.ant_name is not None
            and w.ant_name.startswith(pfx)
        )
    ]


def _split_all_waits(nc):
    """Rewrite every instruction carrying >_MAX_WAITS sem-waits: same-sem
    waits merge to their max threshold, own-engine-clock waits drop (implied
    by program order); remaining excess waits move to same-engine NoOps
    inserted immediately before it (in-order per engine => all waits still
    satisfied before the instruction executes)."""
    for f in nc.m.functions:
        for bb in f.blocks:
            new_insts = []
            for inst in bb.instructions:
                si = inst.sync_info
                waits = _coalesce_waits(list(si.on_wait)) if si else []
                waits = _drop_self_waits(inst, waits)
                if si and len(waits) != len(si.on_wait):
                    inst.sync_info = mybir.SyncInfo(
                        on_wait=waits, on_update=list(si.on_update)
                    )
                if len(waits) > _MAX_WAITS:
                    extra = waits[:-_MAX_WAITS]
                    for i in range(0, len(extra), _MAX_WAITS):
                        new_insts.append(
                            mybir.InstNoOp(
                                name=f"{inst.name}-w{i}",
                                engine=inst.engine,
                                bass_nofuse=True,
                                sync_info=mybir.SyncInfo(
                                    on_wait=extra[i : i + _MAX_WAITS], on_update=[]
                                ),
                            )
                        )
                    inst.sync_info = mybir.SyncInfo(
                        on_wait=waits[-_MAX_WAITS:], on_update=list(si.on_update)
                    )
                new_insts.append(inst)
            bb.instructions = new_insts


# ---------------------------------------------------------------------------
# Device program (identical for all cores; per-core data differs via inputs).
# ---------------------------------------------------------------------------
def _build_program(loop_k: int = 0, exp_off: bool = False):
    nc = bass.Bass("TRN2", target_bir_lowering=False, debug=False)

    d_xs = [nc.dram_tensor(f"xs{j}", [C, PSUB], F32, kind="ExternalInput").ap() for j in (0, 1)]
    d_pos = [nc.dram_tensor(f"posb{j}", [128, NT * 128], F32, kind="ExternalInput").ap() for j in (0, 1)]
    d_wqT = nc.dram_tensor("wqT", [C, C], F32, kind="ExternalInput").ap()
    d_wkT = nc.dram_tensor("wkT", [C, C], F32, kind="ExternalInput").ap()
    d_wvT = nc.dram_tensor("wvT", [C, C], F32, kind="ExternalInput").ap()
    d_bqB = nc.dram_tensor("bqB", [128, C], F32, kind="ExternalInput").ap()
    d_bv2 = nc.dram_tensor("bv2", [128, 2], F32, kind="ExternalInput").ap()
    d_ident = nc.dram_tensor("ident", [128, 128], F32, kind="ExternalInput").ap()
    d_out = nc.dram_tensor("out", [2, S, HS], F32, kind="ExternalOutput").ap()

    with tile.TileContext(nc) as tc:
        import contextlib

        with contextlib.ExitStack() as ctx:
            const = ctx.enter_context(tc.tile_pool(name="const", bufs=1))
            xs_pool = ctx.enter_context(tc.tile_pool(name="xs", bufs=2))
            pos_pool = ctx.enter_context(tc.tile_pool(name="pos", bufs=2))
            qkp_pool = ctx.enter_context(tc.tile_pool(name="qkp", bufs=2))
            yv_pool = ctx.enter_context(tc.tile_pool(name="yv", bufs=2))
            e_pool = ctx.enter_context(tc.tile_pool(name="epool", bufs=4))
            csb_pool = ctx.enter_context(tc.tile_pool(name="csb", bufs=2))
            rc_pool = ctx.enter_context(tc.tile_pool(name="rc", bufs=4))
            osb_pool = ctx.enter_context(tc.tile_pool(name="osb", bufs=4))
            # aux_ps serves the phase-1 projection tiles AND the phase-2
            # transpose tiles (disjoint lifetimes; max tile 1KB) so that the
            # L pool can triple-buffer: aux 2x1KB + L 3x4KB + cps 2KB = 16KB.
            aux_ps = ctx.enter_context(tc.tile_pool(name="auxps", bufs=2, space="PSUM"))
            proj_ps = aux_ps
            tr_ps = aux_ps
            l_ps = ctx.enter_context(tc.tile_pool(name="lps", bufs=2, space="PSUM"))
            ctx_ps = ctx.enter_context(tc.tile_pool(name="ctxps", bufs=2, space="PSUM"))

            # --- constants ---
            wT_sb = {}
            for nm, dt_ in (("wqT", d_wqT), ("wkT", d_wkT), ("wvT", d_wvT)):
                t = const.tile([128, 2 * C], F32, name=f"{nm}_sb")
                nc.sync.dma_start(t[:, 0:C], dt_[0:128, :])
                nc.sync.dma_start(t[:, C : 2 * C], dt_[128:256, :])
                tr = const.tile([128, 2 * C], FP16, name=f"{nm}_h")
                nc.vector.tensor_copy(tr[:, :], t[:, :])
                wT_sb[nm] = tr
            ident_f = const.tile([128, 128], F32, name="ident_f")
            nc.sync.dma_start(ident_f[:, :], d_ident[:, :])
            ident_sb = const.tile([128, 128], BF16, name="ident_sb")
            nc.vector.tensor_copy(ident_sb[:, :], ident_f[:, :])
            bqB_sb = const.tile([128, C], F32, name="bqB_sb")
            nc.sync.dma_start(bqB_sb[:, :], d_bqB[:, :])
            bv_sb = const.tile([128, 2], F32, name="bv_sb")
            nc.sync.dma_start(bv_sb[:, :], d_bv2[:, :])

            # warm the ACT exp table early
            warm = const.tile([128, 8], F32, name="warm")
            nc.gpsimd.memset(warm[:, :], 0.0)
            warm2 = const.tile([128, 8], F32, name="warm2")
            nc.scalar.activation(warm2[:, :], warm[:, :], mybir.ActivationFunctionType.Exp)
            e_const = None
            if exp_off:
                # bench-only: PV consumes a constant E; exp runs at 1/128 width
                e_const = const.tile([128, 1024], BF16, name="e_const")
                nc.gpsimd.memset(e_const[:, :], 0.5)

            Q_sb, KP_sb, YV = [], [], []

            loop_cm = tc.For_i(0, loop_k, 1) if loop_k else contextlib.nullcontext()
            with loop_cm:
              if True:

                # ---------------- phase 1: projections ----------------
                # pair 0 is emitted up front (its KP/Q gate the first flat
                # iterations); pair 1's projection steps are queued as
                # closures and interleaved into the flat loop's PE slack
                # (deadline: flat iteration 128).
                proj_steps = []
                xs_sbs, pos_sbs = [], []
                for bn in (0, 1):
                    xs_f = xs_pool.tile([128, 2 * PSUB], F32, tag="xsf", name=f"xs_f{bn}")
                    nc.sync.dma_start(xs_f[:, 0:PSUB], d_xs[bn][0:128, :])
                    nc.sync.dma_start(xs_f[:, PSUB : 2 * PSUB], d_xs[bn][128:256, :])
                    xs_sb = xs_pool.tile([128, 2 * PSUB], FP16, tag="xsb", name=f"xs_sb{bn}")
                    nc.vector.tensor_copy(xs_sb[:, :], xs_f[:, :])
                    xs_sbs.append(xs_sb)

                    pos_sb = pos_pool.tile([128, NT * 128], F32, name=f"pos_sb{bn}")
                    nc.sync.dma_start(pos_sb[:, :], d_pos[bn][:, :])
                    pos_sbs.append(pos_sb)

                    Q_sb.append(qkp_pool.tile([128, S], FP16, tag="q", name=f"q{bn}"))
                    KP_sb.append(qkp_pool.tile([128, NT * 128], FP16, tag="kp", name=f"kp{bn}"))
                    YV.append(
                        yv_pool.tile([128, 2 * NT * (HS + 1)], BF16, name=f"yv65_{bn}")
                    )

                for bn in (0, 1):
                    xs_sb, pos_sb = xs_sbs[bn], pos_sbs[bn]
                    Q, KP = Q_sb[bn], KP_sb[bn]
                    # Q columns in (t, c) order: query block sb only needs
                    # Q-steps t=2sb,2sb+1, so all but the first two Q-steps
                    # defer into the flat loop (host un-permutes the output).
                    Qv = Q.rearrange("d (t c) -> d t c", c=C)

                    def emit_q_t(t, xs_sb=xs_sb, Qv=Qv):
                        # Q^T projection, replicated to both partition halves by
                        # two col-tiled matmuls (concurrent in the PE array).
                        psq = proj_ps.tile([128, C], F32, tag="pps", name="psq")
                        for half in (0, 1):
                            for kc in (0, 1):
                                nc.tensor.matmul(
                                    psq[64 * half : 64 * half + 64, :],
                                    lhsT=xs_sb[:, kc * PSUB + t * W : kc * PSUB + t * W + W],
                                    rhs=wT_sb["wqT"][:, kc * C : (kc + 1) * C],
                                    start=(kc == 0),
                                    stop=(kc == 1),
                                )
                        nc.vector.tensor_add(Qv[:, t, :], psq[:, :], bqB_sb[:, :])

                    def emit_k_t(t, xs_sb=xs_sb, pos_sb=pos_sb, KP=KP):
                        # K^T projection: chunk A (c<128) -> psum rows 0:64,
                        # chunk B (c>=128) -> psum rows 64:128 (col-tiled).
                        psk = proj_ps.tile([128, 128], F32, tag="pps", name="psk")
                        for half in (0, 1):
                            for kc in (0, 1):
                                nc.tensor.matmul(
                                    psk[64 * half : 64 * half + 64, :],
                                    lhsT=xs_sb[:, kc * PSUB + t * W : kc * PSUB + t * W + W],
                                    rhs=wT_sb["wkT"][:, kc * C + 128 * half : kc * C + 128 * half + 128],
                                    start=(kc == 0),
                                    stop=(kc == 1),
                                )
                        nc.vector.tensor_add(
                            KP[:, t * 128 : (t + 1) * 128], psk[:, :], pos_sb[:, t * 128 : (t + 1) * 128]
                        )

                    # V projection: Yv [c, p] with interleaved ones columns:
                    # yv65 cols = (oc 2) x (t 16) x (64 data + 1 one); key order (t, c)
                    yv65 = YV[bn]
                    yv65_r = yv65.rearrange("p (oc t c) -> p oc t c", oc=2, c=HS + 1)

                    def emit_v_chunk(oc, ph, xs_sb=xs_sb, yv65_r=yv65_r):
                        psv = proj_ps.tile([128, 256], F32, tag="pps", name="psv")
                        for kc in (0, 1):
                            nc.tensor.matmul(
                                psv[:, :],
                                lhsT=wT_sb["wvT"][:, kc * C + oc * 128 : kc * C + oc * 128 + 128],
                                rhs=xs_sb[:, kc * PSUB + ph * 256 : kc * PSUB + ph * 256 + 256],
                                start=(kc == 0),
                                stop=(kc == 1),
                            )
                        nc.vector.tensor_scalar_add(
                            yv65_r[:, oc, 4 * ph : 4 * ph + 4, 0:HS],
                            psv[:, :].rearrange("p (t w) -> p t w", t=4),
                            bv_sb[:, oc : oc + 1],
                        )

                    nc.vector.memset(yv65_r[:, :, :, HS : HS + 1], 1.0)

                    if bn == 0:
                        # minimal prefix before the flat loop: Q-steps for
                        # query block 0 + all K (every m of sb0 needs them)
                        # + all V; Q t>=2 defers (needed from iteration 15 on)
                        emit_q_t(0)
                        emit_q_t(1)
                        for t in range(NT):
                            emit_k_t(t)
                        for oc in (0, 1):
                            for ph in range(4):
                                emit_v_chunk(oc, ph)
                        proj_steps = [lambda t=t, f=emit_q_t: f(t) for t in range(2, NT)]
                    else:
                        proj_steps += [lambda t=t, f=emit_q_t: f(t) for t in range(NT)]
                        proj_steps += [lambda t=t, f=emit_k_t: f(t) for t in range(NT)]
                        proj_steps += [
                            lambda oc=oc, ph=ph, f=emit_v_chunk: f(oc, ph)
                            for oc in (0, 1)
                            for ph in range(4)
                        ]

                # ---------------- phase 2: attention ----------------
                # Flat software-pipelined loop over (pair, sb, m). QK(k+1) is
                # emitted BEFORE PV(k): PE (in-order) fills L(k+1) while ACT
                # exps L(k), so ACT never waits on the PV->QK serial chain.
                # Output stages are deferred one iteration (cps double-buffered).
                yv_r = [
                    YV[bn].rearrange("p (oc t c) -> p oc t c", oc=2, c=HS + 1)
                    for bn in (0, 1)
                ]
                idx = [(bn, sb, m) for bn in (0, 1) for sb in range(8) for m in range(NT)]

                def emit_qk(k):
                    bn, sb, m = idx[k]
                    Q, KP = Q_sb[bn], KP_sb[bn]
                    L = l_ps.tile([128, 1024], F32, name="L")
                    for half in (0, 1):
                        nc.tensor.matmul(
                            L[:, half * 512 : (half + 1) * 512],
                            lhsT=KP[64 * half : 64 * half + 64, m * 128 : (m + 1) * 128],
                            rhs=Q[64 * half : 64 * half + 64, sb * 512 : (sb + 1) * 512],
                            start=True,
                            stop=True,
                        )
                    return L

                def emit_out(bn, sb, csb):
                    for sc in range(4):
                        trp = tr_ps.tile([128, 128], BF16, tag="pps", name="trp")
                        nc.tensor.transpose(
                            trp[:, 0 : HS + 1],
                            csb[:, sc * 128 : (sc + 1) * 128],
                            ident_sb[0 : HS + 1, 0 : HS + 1],
                        )
                        rc = rc_pool.tile([128, 1], F32, name="rcp")
                        nc.vector.reciprocal(rc[:, 0:1], trp[:, HS : HS + 1])
                        osb = osb_pool.tile([128, HS], F32, name="osb")
                        nc.vector.tensor_scalar_mul(osb[:, :], trp[:, 0:HS], rc[:, 0:1])
                        r0 = sb * 512 + sc * 128
                        nc.sync.dma_start(d_out[bn, r0 : r0 + 128, :], osb[:, :])

                L_next = emit_qk(0)
                cps = None
                pending_out = None
                for k, (bn, sb, m) in enumerate(idx):
                    if m == 0:
                        cps = ctx_ps.tile([HS + 1, 512], F32, name="cps")
                    L = L_next
                    E = e_pool.tile([128, 1024], BF16, name="E")
                    if exp_off:
                        nc.scalar.activation(
                            E[:, 0:8], L[:, 0:8], mybir.ActivationFunctionType.Exp
                        )
                        E = e_const
                    else:
                        nc.scalar.activation(
                            E[:, :], L[:, :], mybir.ActivationFunctionType.Exp
                        )

                    # PE order QK(k+1), PV(k): QK(k+1) has no dependency on
                    # exp(k), so PE fills L(k+1) while ACT runs exp(k); only
                    # PV(k) waits on exp(k). ACT sees L(k+1) ready early and
                    # runs back-to-back.
                    if k + 1 < len(idx):
                        L_next = emit_qk(k + 1)
                    for half in (0, 1):
                        nc.tensor.matmul(
                            cps[:, :],
                            lhsT=yv_r[bn][:, half, m, :],
                            rhs=E[:, half * 512 : (half + 1) * 512],
                            start=(m == 0 and half == 0),
                            stop=(m == NT - 1 and half == 1),
                        )
                    if pending_out is not None:
                        emit_out(*pending_out)
                        pending_out = None
                    # interleave one pair-1 projection step into the PE slack
                    # every other iteration (40 steps, done by iteration ~80)
                    if k % 2 == 0 and proj_steps:
                        proj_steps.pop(0)()
                    if m == NT - 1:
                        csb = csb_pool.tile([HS + 1, 512], BF16, name="csb")
                        nc.vector.tensor_copy(csb[:, :], cps[:, :])
                        pending_out = (bn, sb, csb)
                if pending_out is not None:
                    emit_out(*pending_out)

    _split_all_waits(nc)
    return nc


_NC = None


def _get_nc():
    global _NC
    if _NC is None:
        _NC = _build_program()
    return _NC


def build_in_maps(x, wq, bq, wk, bk, wv, bv, rel_h, rel_w):
    x = np.ascontiguousarray(np.asarray(x, np.float32))
    wq, wk, wv = (np.asarray(a, np.float32) for a in (wq, wk, wv))
    bq, bk, bv = (np.asarray(a, np.float32) for a in (bq, bk, bv))
    rel_h = np.asarray(rel_h, np.float32)
    rel_w = np.asarray(rel_w, np.float32)

    wqT = np.ascontiguousarray(wq.T)
    wkT = np.ascontiguousarray(wk.T)
    wvT = np.ascontiguousarray(wv.T)
    bqB = np.ascontiguousarray(np.broadcast_to(bq[None, :], (128, C)))
    bv2 = np.ascontiguousarray(bv.reshape(2, 128).T)  # bv2[p, oc] = bv[oc*128 + p]
    ident = np.eye(128, dtype=np.float32)
    cols = np.arange(S)

    in_maps = []
    for core in range(N_CORES):
        m = {
            "wqT": wqT, "wkT": wkT, "wvT": wvT,
            "bqB": bqB, "bv2": bv2, "ident": ident,
        }
        for j in (0, 1):
            p = 2 * core + j
            b, n = p // NH, p % NH
            m[f"xs{j}"] = np.ascontiguousarray(x[b, :, n::4, :]).reshape(C, PSUB)
            rw = rel_w[0, n, :, :, 0]  # [hs, W]
            rh = rel_h[0, n, :, 0, :]  # [hs, H]
            pos = (rw[:, :, None] + rh[:, None, :]).reshape(HS, S)
            pb = pos + bk[cols // NT][None, :]
            # key axis (c, t) -> (t, c): pbt[d, t*256 + c]
            pbt = pb.reshape(HS, C, NT).transpose(0, 2, 1)  # [hs, t, c]
            # split c into halves; stack on the partition axis:
            # rows 0:64 = keys (t, c<128), rows 64:128 = keys (t, c>=128)
            posb = np.empty((128, NT * 128), np.float32)
            posb[0:HS] = pbt[:, :, 0:128].reshape(HS, NT * 128)
            posb[HS:128] = pbt[:, :, 128:256].reshape(HS, NT * 128)
            m[f"posb{j}"] = np.ascontiguousarray(posb)
        in_maps.append(m)
    return in_maps


_R = np.arange(S)
# device output rows are queries in (t, c) order; token s = c*16 + t
_S_OF_R = (_R % C) * NT + (_R // C)


def assemble_output(results):
    out_full = np.empty((NH, S, NH, HS), np.float32)  # [B, S, nh, hs]
    for core in range(N_CORES):
        o = results[core]["out"]  # [2, S, HS]
        for j in (0, 1):
            p = 2 * core + j
            b, n = p // NH, p % NH
            out_full[b, _S_OF_R, n, :] = o[j]
    return out_full.reshape(NH, S * NH * HS).reshape(NH, C, W, H)


def kernel(x, wq, bq, wk, bk, wv, bv, rel_h, rel_w):
    in_maps = build_in_maps(x, wq, bq, wk, bk, wv, bv, rel_h, rel_w)
    res = run_bass_kernel_spmd(_get_nc(), in_maps, core_ids=list(range(N_CORES)))
    return assemble_output(res.results)
